# revision 12
# baseline (speedup 1.0000x reference)
"""Trainium2 Bass kernel for bag-level attention (ragged_sequence).

Math (per bag b over its 16 sentences i):
    att_i  = <x_i, rel[q_i]>
    w      = softmax(att) within bag
    logits = (sum_i w_i x_i) @ rel.T + bias

Key identity: logits[b] = sum_i w_i S[i,:] + bias with S = x @ rel.T, so x is
read from HBM exactly once. The kernel is HBM-bound; x is streamed as plain
fp16 (the 2e-2 correctness gate leaves ~50x margin at fp16's ~3.5e-4 rel err),
which halves traffic vs an fp16 hi+lo split.

Engine budget notes (measured): DVE/ScalarE cost ~1.2ns per FREE element
regardless of partition count, and 16-bit operands run ~3x faster, so the
whole softmax chain runs in fp16 and the PSUM read (the only fp32-rate pass)
is done once, on the otherwise-idle ScalarE. PE matmuls re-use each rel
k-chunk across the 4 compute chunks of a superchunk (k-outer order) since
back-to-back same-stationary matmuls stream ~1.6x faster.

Device layout (per core, rows = N/8 sentences, ch=512-sentence chunks):
    st[64, ch]   = relT(64-col zero-padded).T @ xT chunk     (PSUM, 6 matmuls)
    s16[53, ch]  = fp16 copy of st                           (ScalarE)
    oh[53, sc]   = one-hot of query: bcast(q) == iota        (GpSimd + DVE)
    sm           = s16 * oh (fp16)                           (DVE)
    att[1, ch]   = ones53.T @ sm  (4 per sc share one PSUM   (PE)
                   bank at partitions 0/32/64/96)
    e16          = exp(att) fp16                             (ScalarE)
    zac row      = windowed reduce_16(e16)                   (DVE)
    ebs[53, ch]  = partition_broadcast(e16)                  (GpSimd)
    ltz rows     = windowed reduce_16(s16 * ebs)             (DVE x2, fp16)
  Every 128 bags: pt[128, 54] = ltz_blk.T @ bt53 + zac_blk.T @ btz gives
  pt[:, c] = lu[c] + z*bias[c] and pt[:, 53] = z; then
  logits_block = pt[:, 0:53] * (1/z) per-partition (3 ops per 128 bags).
"""

import os
from contextlib import ExitStack

import numpy as np

import concourse.bass as bass
import concourse.tile as tile
from concourse import bacc, library_config, mybir
from concourse.bass_utils import run_bass_kernel_spmd

# Problem constants (hardcoded per spec nn_Attention_85478439125349)
N = 262144
B = 16384
D = 768
C = 53
BAG = 16
N_CORES = 8
ROWS = N // N_CORES          # 32768 sentences per core
BAGS = B // N_CORES          # 2048 bags per core
KCH = D // 128               # 6 contraction chunks
F32 = mybir.dt.float32
F16 = mybir.dt.float16


def build_nc(rows: int, sc: int = 2048, ch: int = 512) -> bass.Bass:
    """Build the per-core Bass program for `rows` sentences (bags of BAG)."""
    assert rows % sc == 0 and sc % ch == 0 and ch % BAG == 0
    bags = rows // BAG
    n_sc = rows // sc          # superchunks (DMA granularity)
    n_ch = sc // ch            # compute chunks per superchunk
    chb = ch // BAG            # bags per compute chunk (32)

    nc = bacc.Bacc()
    # x fp16, partition-major packed per superchunk so each partition's
    # DMA run is KCH*sc contiguous elements: xt3[p, isc, k, j] =
    # xT[128k+p, isc*sc+j]
    xt3 = nc.declare_dram_parameter(
        "xt3", [128, rows // sc, KCH, sc], F16, isOutput=False
    )
    # query as fp16 row (values 0..52, exact in fp16)
    qft = nc.declare_dram_parameter("qft", [1, rows], F16, isOutput=False)
    # relT zero-padded to 64 output columns, split by contraction chunk
    relt = nc.declare_dram_parameter("relt", [128, KCH, 64], F16, isOutput=False)
    iotat = nc.declare_dram_parameter("iotat", [64, 1], F32, isOutput=False)
    onest = nc.declare_dram_parameter("onest", [C, 1], F16, isOutput=False)
    onesrt = nc.declare_dram_parameter("onesrt", [1, 65], F16, isOutput=False)
    # Augmented transpose operand: pt = ltz_blk.T @ bt65 emits
    # lu + z*bias in cols 0:53 and z itself in col 53 (ltz row 64 holds z
    # thanks to the ones-row in s16; bt65 row 64 = [bias..., 1]).
    bt65m = nc.declare_dram_parameter("bt65m", [65, C + 1], F32, isOutput=False)
    out = nc.declare_dram_parameter("out", [bags, C], F32, isOutput=True)

    with tile.TileContext(nc) as tc, ExitStack() as ctx:
        consts = ctx.enter_context(tc.tile_pool(name="consts", bufs=1))
        xpool = ctx.enter_context(tc.tile_pool(name="xpool", bufs=3))
        ohpool = ctx.enter_context(tc.tile_pool(name="ohpool", bufs=2))
        work = ctx.enter_context(tc.tile_pool(name="work", bufs=3))
        psum = ctx.enter_context(tc.tile_pool(name="psum", bufs=2, space="PSUM"))

        # --- constants ---
        relt_sb = consts.tile([128, KCH, 64], F16)
        nc.sync.dma_start(out=relt_sb, in_=relt[:, :, :])
        iota_sb = consts.tile([64, 1], F32)
        nc.sync.dma_start(out=iota_sb, in_=iotat[:, :])
        ones_sb = consts.tile([C, 1], F16)
        nc.sync.dma_start(out=ones_sb, in_=onest[:, :])
        onesrow_sb = consts.tile([1, 65], F16)
        nc.sync.dma_start(out=onesrow_sb, in_=onesrt[:, :])
        bt65_sb = consts.tile([65, C + 1], F32)
        nc.sync.dma_start(out=bt65_sb, in_=bt65m[:, :])
        nc.gpsimd.load_library(library_config.attn)
        # ltz rows 0:53 accumulate unnormalized logits^T, row 64 the bag
        # sums z (rows 53:64 are zero padding from rel's zero columns)
        ltz = consts.tile([65, bags], F32)
        logits_sb = consts.tile([128, bags // 128, C], F32)

        # s16 ring: 6 persistent [65, ch] fp16 tiles whose ones-row at
        # partition 64 is written ONCE here; thereafter the windowed reduce
        # of w16 = s16 * ebs yields z in row 64 for free.
        s16ring = [
            consts.tile([65, ch], F16, name=f"s16r{j}") for j in range(6)
        ]
        for t in s16ring:
            nc.vector.memset(t[64:65, :], 1.0)

        # Chunk-granular software pipeline (per-engine in-order queues stay
        # interleaved): chunk i's matmuls issue while chunk i-1's softmax
        # and chunk i-2's weighted reduce run.
        pend_a = {}  # i -> (st, sm): waiting for att/exp/ebs stage
        pend_b = {}  # i -> (s16, ebs): waiting for weighted-sum stage

        def stage_mid(i):
            st, sm = pend_a.pop(i)
            # att pairs share one PSUM bank at partition offsets 0/64; the
            # odd ones sit on PE tile (0,64) whose weight load can overlap
            if i % 2 == 0:
                stage_mid.att2 = psum.tile([65, ch], F32, tag="att2", bufs=2)
            att = stage_mid.att2[64 * (i % 2) : 64 * (i % 2) + 1, :]
            nc.tensor.matmul(
                att,
                lhsT=ones_sb,
                rhs=sm,
                start=True,
                stop=True,
                skip_group_check=(i % 2 > 0),
            )
            e16 = work.tile([1, ch], F16, tag="e16", bufs=4)
            nc.scalar.activation(e16, att, mybir.ActivationFunctionType.Exp)
            # broadcast e across 65 partitions on the PE (contraction-1
            # matmul into PSUM) instead of the slow GpSimd broadcast
            ebs = psum.tile([65, ch], F32, tag="ebs", bufs=2)
            nc.tensor.matmul(ebs, lhsT=onesrow_sb, rhs=e16, start=True, stop=True)
            pend_b[i] = (s16ring[i % 6], ebs)

        def stage_late(i):
            s16, ebs = pend_b.pop(i)
            w16 = work.tile([65, ch], F16, tag="w16", bufs=3)
            nc.vector.tensor_mul(w16, s16, ebs)
            ob = i * chb
            # one windowed reduce emits the logit rows (0:53) and, via the
            # ones-row at partition 64 of s16, the bag sums z (row 64)
            nc.vector.reduce_sum(
                ltz[:, ob : ob + chb],
                w16.rearrange("p (b j) -> p b j", j=BAG),
                axis=mybir.AxisListType.X,
            )
            # transpose each completed 128-bag block to [bags, C], normalize
            if (i + 1) * chb % 128 == 0:
                t = ((i + 1) * chb) // 128 - 1
                pt = psum.tile([128, C + 1], F32, tag="pt", bufs=2)
                nc.tensor.matmul(
                    pt,
                    lhsT=ltz[:, t * 128 : (t + 1) * 128],
                    rhs=bt65_sb,
                    start=True,
                    stop=True,
                )
                rzc = work.tile([128, 1], F32, tag="rzc")
                nc.vector.reciprocal(rzc, pt[:, C : C + 1])
                nc.vector.tensor_scalar_mul(
                    out=logits_sb[:, t, :], in0=pt[:, 0:C], scalar1=rzc
                )

        x_sb = oh_sb = None
        for i in range(n_sc * n_ch):
            s, ic = divmod(i, n_ch)
            if ic == 0:
                x_sb = xpool.tile([128, KCH, sc], F16, bufs=4)
                kh = KCH // 2
                nc.sync.dma_start(out=x_sb[:, 0:kh, :], in_=xt3[:, s, 0:kh, :])
                nc.sync.dma_start(
                    out=x_sb[:, kh:KCH, :], in_=xt3[:, s, kh:KCH, :]
                )
                qf_sb = xpool.tile([1, sc], F16, tag="qf", bufs=4)
                nc.sync.dma_start(
                    out=qf_sb, in_=qft[:, s * sc : (s + 1) * sc]
                )
                qb_sb = ohpool.tile([64, sc], F16, tag="qb", bufs=2)
                nc.gpsimd.partition_broadcast(qb_sb, qf_sb, channels=64)
                oh_sb = ohpool.tile([64, sc], F16, tag="oh", bufs=2)
                nc.vector.tensor_scalar(
                    out=oh_sb,
                    in0=qb_sb,
                    scalar1=iota_sb,
                    scalar2=None,
                    op0=mybir.AluOpType.is_equal,
                )

            cs = slice(ic * ch, (ic + 1) * ch)
            st = psum.tile([64, ch], F32, tag="st", bufs=2)
            for k in range(KCH):
                nc.tensor.matmul(
                    st,
                    lhsT=relt_sb[:, k, :],
                    rhs=x_sb[:, k, cs],
                    start=(k == 0),
                    stop=(k == KCH - 1),
                )
            # fp16 copy of st into the ring (frees the PSUM bank quickly);
            # ScalarE is otherwise idle
            s16 = s16ring[i % 6]
            nc.scalar.activation(
                s16[0:64, :], st, mybir.ActivationFunctionType.Copy
            )
            sm = work.tile([C, ch], F16, tag="sm", bufs=4)
            nc.vector.tensor_mul(sm, st[0:C, :], oh_sb[0:C, cs])
            pend_a[i] = (st, sm)
            if i > 0:
                stage_mid(i - 1)
            if i > 1:
                stage_late(i - 2)
        n_total = n_sc * n_ch
        stage_mid(n_total - 1)
        stage_late(n_total - 2)
        stage_late(n_total - 1)
        nc.sync.dma_start(
            out=out.rearrange("(t p) c -> p t c", p=128), in_=logits_sb
        )
    return nc


_NC_CACHE: dict = {}


def _get_nc(rows: int) -> bass.Bass:
    if rows not in _NC_CACHE:
        nc = build_nc(rows)
        nc.finalize()
        _NC_CACHE[rows] = nc
    return _NC_CACHE[rows]


def _numpy_fallback(x, rel_weight, bias, input_scope, query):
    """Pure-numpy replication of the reference for non-uniform bag layouts."""
    n = x.shape[0]
    num_bags = input_scope.shape[0] - 1
    seg = np.searchsorted(input_scope[1:], np.arange(n), side="right")
    att = np.einsum("nd,nd->n", x, rel_weight[query]).astype(np.float32)
    valid = seg < num_bags
    segv = seg[valid]
    attv = att[valid]
    m = np.full(num_bags, -np.inf, dtype=np.float32)
    np.maximum.at(m, segv, attv)
    e = np.zeros(n, dtype=np.float32)
    e[valid] = np.exp(attv - m[segv])
    z = np.zeros(num_bags, dtype=np.float32)
    np.add.at(z, segv, e[valid])
    w = np.zeros(n, dtype=np.float32)
    nz = z[segv] != 0
    w_valid = np.zeros(segv.shape[0], dtype=np.float32)
    w_valid[nz] = e[valid][nz] / z[segv][nz]
    w[valid] = w_valid
    repre = np.zeros((num_bags, x.shape[1]), dtype=np.float32)
    np.add.at(repre, segv, (x[valid] * w[valid][:, None]).astype(np.float32))
    return repre @ rel_weight.T + bias


def _pack_x(xt_h, sc):
    """[D, rows] fp16 -> [128, rows//sc, KCH, sc] so each partition's
    per-superchunk DMA run (KCH*sc elements) is contiguous."""
    rows = xt_h.shape[1]
    v = xt_h.reshape(KCH, 128, rows // sc, sc)
    return np.ascontiguousarray(v.transpose(1, 2, 0, 3))


def _prepare_in_maps(x, rel_weight, bias, query, sc=2048):
    relp = np.zeros((D, 64), dtype=np.float16)
    relp[:, :C] = rel_weight.T.astype(np.float16)
    relt = np.ascontiguousarray(
        relp.reshape(KCH, 128, 64).transpose(1, 0, 2)
    )
    iotat = np.arange(64, dtype=np.float32).reshape(64, 1)
    onest = np.ones((C, 1), dtype=np.float16)
    onesrt = np.ones((1, 65), dtype=np.float16)
    bt65m = np.zeros((65, C + 1), dtype=np.float32)
    bt65m[np.arange(C), np.arange(C)] = 1.0
    bt65m[64, :C] = bias.astype(np.float32)
    bt65m[64, C] = 1.0
    q = query.astype(np.float16).reshape(1, -1)
    in_maps = []
    for c in range(N_CORES):
        lo_r, hi_r = c * ROWS, (c + 1) * ROWS
        xh = x[lo_r:hi_r].astype(np.float16)
        in_maps.append(
            {
                "xt3": _pack_x(np.ascontiguousarray(xh.T), sc),
                "qft": np.ascontiguousarray(q[:, lo_r:hi_r]),
                "relt": relt,
                "iotat": iotat,
                "onest": onest,
                "onesrt": onesrt,
                "bt65m": bt65m,
            }
        )
    return in_maps


def run_device(x, rel_weight, bias, query, trace=False, **kwargs):
    nc = _get_nc(ROWS)
    in_maps = _prepare_in_maps(x, rel_weight, bias, query)
    res = run_bass_kernel_spmd(
        nc, in_maps, core_ids=list(range(N_CORES)), trace=trace, **kwargs
    )
    outs = [np.asarray(r["out"]) for r in res.results]
    return np.concatenate(outs, axis=0), res


def kernel(x, rel_weight, bias, input_scope, query):
    x = np.asarray(x, dtype=np.float32)
    rel_weight = np.asarray(rel_weight, dtype=np.float32)
    bias = np.asarray(bias, dtype=np.float32)
    input_scope = np.asarray(input_scope)
    query = np.asarray(query)

    expected_scope = np.arange(B + 1, dtype=np.int64) * (N // B)
    if (
        x.shape == (N, D)
        and rel_weight.shape == (C, D)
        and input_scope.shape == (B + 1,)
        and np.array_equal(input_scope.astype(np.int64), expected_scope)
    ):
        out, _ = run_device(x, rel_weight, bias, query)
        return out
    return _numpy_fallback(x, rel_weight, bias, input_scope, query)


# revision 13
# speedup vs baseline: 1.0039x; 1.0039x over previous
"""Trainium2 Bass kernel for bag-level attention (ragged_sequence).

Math (per bag b over its 16 sentences i):
    att_i  = <x_i, rel[q_i]>
    w      = softmax(att) within bag
    logits = (sum_i w_i x_i) @ rel.T + bias

Key identity: logits[b] = sum_i w_i S[i,:] + bias with S = x @ rel.T, so x is
read from HBM exactly once. The kernel is HBM-bound; x is streamed as plain
fp16 (the 2e-2 correctness gate leaves ~50x margin at fp16's ~3.5e-4 rel err),
which halves traffic vs an fp16 hi+lo split.

Engine budget notes (measured): DVE/ScalarE cost ~1.2ns per FREE element
regardless of partition count, and 16-bit operands run ~3x faster, so the
whole softmax chain runs in fp16 and the PSUM read (the only fp32-rate pass)
is done once, on the otherwise-idle ScalarE. PE matmuls re-use each rel
k-chunk across the 4 compute chunks of a superchunk (k-outer order) since
back-to-back same-stationary matmuls stream ~1.6x faster.

Device layout (per core, rows = N/8 sentences, ch=512-sentence chunks):
    st[64, ch]   = relT(64-col zero-padded).T @ xT chunk     (PSUM, 6 matmuls)
    s16[53, ch]  = fp16 copy of st                           (ScalarE)
    oh[53, sc]   = one-hot of query: bcast(q) == iota        (GpSimd + DVE)
    sm           = s16 * oh (fp16)                           (DVE)
    att[1, ch]   = ones53.T @ sm  (4 per sc share one PSUM   (PE)
                   bank at partitions 0/32/64/96)
    e16          = exp(att) fp16                             (ScalarE)
    zac row      = windowed reduce_16(e16)                   (DVE)
    ebs[53, ch]  = partition_broadcast(e16)                  (GpSimd)
    ltz rows     = windowed reduce_16(s16 * ebs)             (DVE x2, fp16)
  Every 128 bags: pt[128, 54] = ltz_blk.T @ bt53 + zac_blk.T @ btz gives
  pt[:, c] = lu[c] + z*bias[c] and pt[:, 53] = z; then
  logits_block = pt[:, 0:53] * (1/z) per-partition (3 ops per 128 bags).
"""

import os
from contextlib import ExitStack

import numpy as np

import concourse.bass as bass
import concourse.tile as tile
from concourse import bacc, library_config, mybir
from concourse.bass_utils import run_bass_kernel_spmd

# Problem constants (hardcoded per spec nn_Attention_85478439125349)
N = 262144
B = 16384
D = 768
C = 53
BAG = 16
N_CORES = 8
ROWS = N // N_CORES          # 32768 sentences per core
BAGS = B // N_CORES          # 2048 bags per core
KCH = D // 128               # 6 contraction chunks
F32 = mybir.dt.float32
F16 = mybir.dt.float16


def build_nc(rows: int, sc: int = 2048, ch: int = 512) -> bass.Bass:
    """Build the per-core Bass program for `rows` sentences (bags of BAG)."""
    assert rows % sc == 0 and sc % ch == 0 and ch % BAG == 0
    bags = rows // BAG
    n_sc = rows // sc          # superchunks (DMA granularity)
    n_ch = sc // ch            # compute chunks per superchunk
    chb = ch // BAG            # bags per compute chunk (32)

    nc = bacc.Bacc()
    # x fp16, partition-major packed per superchunk so each partition's
    # DMA run is KCH*sc contiguous elements: xt3[p, isc, k, j] =
    # xT[128k+p, isc*sc+j]
    xt3 = nc.declare_dram_parameter(
        "xt3", [128, rows // sc, KCH, sc], F16, isOutput=False
    )
    # query as fp16 row (values 0..52, exact in fp16)
    qft = nc.declare_dram_parameter("qft", [1, rows], F16, isOutput=False)
    # relT zero-padded to 64 output columns, split by contraction chunk
    relt = nc.declare_dram_parameter("relt", [128, KCH, 64], F16, isOutput=False)
    iotat = nc.declare_dram_parameter("iotat", [64, 1], F32, isOutput=False)
    onest = nc.declare_dram_parameter("onest", [C, 1], F16, isOutput=False)
    # Augmented transpose operand: pt = ltz_blk.T @ bt65 emits
    # lu + z*bias in cols 0:53 and z itself in col 53 (ltz row 64 holds z
    # thanks to the ones-row in s16; bt65 row 64 = [bias..., 1]).
    bt65m = nc.declare_dram_parameter("bt65m", [65, C + 1], F32, isOutput=False)
    out = nc.declare_dram_parameter("out", [bags, C], F32, isOutput=True)

    with tile.TileContext(nc) as tc, ExitStack() as ctx:
        consts = ctx.enter_context(tc.tile_pool(name="consts", bufs=1))
        xpool = ctx.enter_context(tc.tile_pool(name="xpool", bufs=3))
        ohpool = ctx.enter_context(tc.tile_pool(name="ohpool", bufs=2))
        work = ctx.enter_context(tc.tile_pool(name="work", bufs=3))
        psum = ctx.enter_context(tc.tile_pool(name="psum", bufs=2, space="PSUM"))

        # --- constants ---
        relt_sb = consts.tile([128, KCH, 64], F16)
        nc.sync.dma_start(out=relt_sb, in_=relt[:, :, :])
        iota_sb = consts.tile([64, 1], F32)
        nc.sync.dma_start(out=iota_sb, in_=iotat[:, :])
        ones_sb = consts.tile([C, 1], F16)
        nc.sync.dma_start(out=ones_sb, in_=onest[:, :])
        bt65_sb = consts.tile([65, C + 1], F32)
        nc.sync.dma_start(out=bt65_sb, in_=bt65m[:, :])
        nc.gpsimd.load_library(library_config.attn)
        # ltz rows 0:53 accumulate unnormalized logits^T, row 64 the bag
        # sums z (rows 53:64 are zero padding from rel's zero columns)
        ltz = consts.tile([65, bags], F32)
        logits_sb = consts.tile([128, bags // 128, C], F32)

        # s16 ring: 6 persistent [65, ch] fp16 tiles whose ones-row at
        # partition 64 is written ONCE here; thereafter the windowed reduce
        # of w16 = s16 * ebs yields z in row 64 for free.
        s16ring = [
            consts.tile([65, ch], F16, name=f"s16r{j}") for j in range(6)
        ]
        for t in s16ring:
            nc.vector.memset(t[64:65, :], 1.0)

        # Chunk-granular software pipeline (per-engine in-order queues stay
        # interleaved): chunk i's matmuls issue while chunk i-1's softmax
        # and chunk i-2's weighted reduce run.
        pend_a = {}  # i -> (st, sm): waiting for att/exp/ebs stage
        pend_b = {}  # i -> (s16, ebs): waiting for weighted-sum stage
        pend_c: list = []  # completed 128-bag blocks awaiting transpose

        def stage_mid(i):
            st, sm = pend_a.pop(i)
            # att pairs share one PSUM bank at partition offsets 0/64; the
            # odd ones sit on PE tile (0,64) whose weight load can overlap
            if i % 2 == 0:
                stage_mid.att2 = psum.tile([65, ch], F32, tag="att2", bufs=2)
            att = stage_mid.att2[64 * (i % 2) : 64 * (i % 2) + 1, :]
            nc.tensor.matmul(
                att,
                lhsT=ones_sb,
                rhs=sm,
                start=True,
                stop=True,
                skip_group_check=(i % 2 > 0),
            )
            e = work.tile([1, ch], F32, tag="e", bufs=4)
            nc.scalar.activation(e, att, mybir.ActivationFunctionType.Exp)
            ebs = work.tile([65, ch], F32, tag="ebs", bufs=3)
            nc.gpsimd.partition_broadcast(ebs, e, channels=65)
            pend_b[i] = (s16ring[i % 6], ebs)

        def stage_late(i):
            s16, ebs = pend_b.pop(i)
            w16 = work.tile([65, ch], F16, tag="w16", bufs=3)
            nc.vector.tensor_mul(w16, s16, ebs)
            ob = i * chb
            # one windowed reduce emits the logit rows (0:53) and, via the
            # ones-row at partition 64 of s16, the bag sums z (row 64)
            nc.vector.reduce_sum(
                ltz[:, ob : ob + chb],
                w16.rearrange("p (b j) -> p b j", j=BAG),
                axis=mybir.AxisListType.X,
            )
            if (i + 1) * chb % 128 == 0:
                pend_c.append(((i + 1) * chb) // 128 - 1)

        def stage_fin():
            # transpose a completed 128-bag block to [bags, C] and
            # normalize; deferred one chunk so the PE never waits on the
            # just-issued reduce
            t = pend_c.pop(0)
            pt = psum.tile([128, C + 1], F32, tag="pt", bufs=2)
            nc.tensor.matmul(
                pt,
                lhsT=ltz[:, t * 128 : (t + 1) * 128],
                rhs=bt65_sb,
                start=True,
                stop=True,
            )
            rzc = work.tile([128, 1], F32, tag="rzc")
            nc.vector.reciprocal(rzc, pt[:, C : C + 1])
            nc.vector.tensor_scalar_mul(
                out=logits_sb[:, t, :], in0=pt[:, 0:C], scalar1=rzc
            )

        x_sb = oh_sb = None
        for i in range(n_sc * n_ch):
            s, ic = divmod(i, n_ch)
            if ic == 0:
                x_sb = xpool.tile([128, KCH, sc], F16, bufs=4)
                kh = KCH // 2
                nc.sync.dma_start(out=x_sb[:, 0:kh, :], in_=xt3[:, s, 0:kh, :])
                nc.sync.dma_start(
                    out=x_sb[:, kh:KCH, :], in_=xt3[:, s, kh:KCH, :]
                )
                qf_sb = xpool.tile([1, sc], F16, tag="qf", bufs=4)
                nc.sync.dma_start(
                    out=qf_sb, in_=qft[:, s * sc : (s + 1) * sc]
                )
                qb_sb = ohpool.tile([64, sc], F16, tag="qb", bufs=2)
                nc.gpsimd.partition_broadcast(qb_sb, qf_sb, channels=64)
                oh_sb = ohpool.tile([64, sc], F16, tag="oh", bufs=2)
                nc.vector.tensor_scalar(
                    out=oh_sb,
                    in0=qb_sb,
                    scalar1=iota_sb,
                    scalar2=None,
                    op0=mybir.AluOpType.is_equal,
                )

            cs = slice(ic * ch, (ic + 1) * ch)
            st = psum.tile([64, ch], F32, tag="st", bufs=3)
            for k in range(KCH):
                nc.tensor.matmul(
                    st,
                    lhsT=relt_sb[:, k, :],
                    rhs=x_sb[:, k, cs],
                    start=(k == 0),
                    stop=(k == KCH - 1),
                )
            # fp16 copy of st into the ring (frees the PSUM bank quickly);
            # ScalarE is otherwise idle
            s16 = s16ring[i % 6]
            nc.scalar.activation(
                s16[0:64, :], st, mybir.ActivationFunctionType.Copy
            )
            sm = work.tile([C, ch], F16, tag="sm", bufs=4)
            nc.vector.tensor_mul(sm, st[0:C, :], oh_sb[0:C, cs])
            pend_a[i] = (st, sm)
            if pend_c:
                stage_fin()
            if i > 0:
                stage_mid(i - 1)
            if i > 1:
                stage_late(i - 2)
        n_total = n_sc * n_ch
        stage_mid(n_total - 1)
        stage_late(n_total - 2)
        stage_late(n_total - 1)
        while pend_c:
            stage_fin()
        nc.sync.dma_start(
            out=out.rearrange("(t p) c -> p t c", p=128), in_=logits_sb
        )
    return nc


_NC_CACHE: dict = {}


def _get_nc(rows: int) -> bass.Bass:
    if rows not in _NC_CACHE:
        nc = build_nc(rows)
        nc.finalize()
        _NC_CACHE[rows] = nc
    return _NC_CACHE[rows]


def _numpy_fallback(x, rel_weight, bias, input_scope, query):
    """Pure-numpy replication of the reference for non-uniform bag layouts."""
    n = x.shape[0]
    num_bags = input_scope.shape[0] - 1
    seg = np.searchsorted(input_scope[1:], np.arange(n), side="right")
    att = np.einsum("nd,nd->n", x, rel_weight[query]).astype(np.float32)
    valid = seg < num_bags
    segv = seg[valid]
    attv = att[valid]
    m = np.full(num_bags, -np.inf, dtype=np.float32)
    np.maximum.at(m, segv, attv)
    e = np.zeros(n, dtype=np.float32)
    e[valid] = np.exp(attv - m[segv])
    z = np.zeros(num_bags, dtype=np.float32)
    np.add.at(z, segv, e[valid])
    w = np.zeros(n, dtype=np.float32)
    nz = z[segv] != 0
    w_valid = np.zeros(segv.shape[0], dtype=np.float32)
    w_valid[nz] = e[valid][nz] / z[segv][nz]
    w[valid] = w_valid
    repre = np.zeros((num_bags, x.shape[1]), dtype=np.float32)
    np.add.at(repre, segv, (x[valid] * w[valid][:, None]).astype(np.float32))
    return repre @ rel_weight.T + bias


def _pack_x(xt_h, sc):
    """[D, rows] fp16 -> [128, rows//sc, KCH, sc] so each partition's
    per-superchunk DMA run (KCH*sc elements) is contiguous."""
    rows = xt_h.shape[1]
    v = xt_h.reshape(KCH, 128, rows // sc, sc)
    return np.ascontiguousarray(v.transpose(1, 2, 0, 3))


def _prepare_in_maps(x, rel_weight, bias, query, sc=2048):
    relp = np.zeros((D, 64), dtype=np.float16)
    relp[:, :C] = rel_weight.T.astype(np.float16)
    relt = np.ascontiguousarray(
        relp.reshape(KCH, 128, 64).transpose(1, 0, 2)
    )
    iotat = np.arange(64, dtype=np.float32).reshape(64, 1)
    onest = np.ones((C, 1), dtype=np.float16)
    bt65m = np.zeros((65, C + 1), dtype=np.float32)
    bt65m[np.arange(C), np.arange(C)] = 1.0
    bt65m[64, :C] = bias.astype(np.float32)
    bt65m[64, C] = 1.0
    q = query.astype(np.float16).reshape(1, -1)
    in_maps = []
    for c in range(N_CORES):
        lo_r, hi_r = c * ROWS, (c + 1) * ROWS
        xh = x[lo_r:hi_r].astype(np.float16)
        in_maps.append(
            {
                "xt3": _pack_x(np.ascontiguousarray(xh.T), sc),
                "qft": np.ascontiguousarray(q[:, lo_r:hi_r]),
                "relt": relt,
                "iotat": iotat,
                "onest": onest,
                "bt65m": bt65m,
            }
        )
    return in_maps


def run_device(x, rel_weight, bias, query, trace=False, **kwargs):
    nc = _get_nc(ROWS)
    in_maps = _prepare_in_maps(x, rel_weight, bias, query)
    res = run_bass_kernel_spmd(
        nc, in_maps, core_ids=list(range(N_CORES)), trace=trace, **kwargs
    )
    outs = [np.asarray(r["out"]) for r in res.results]
    return np.concatenate(outs, axis=0), res


def kernel(x, rel_weight, bias, input_scope, query):
    x = np.asarray(x, dtype=np.float32)
    rel_weight = np.asarray(rel_weight, dtype=np.float32)
    bias = np.asarray(bias, dtype=np.float32)
    input_scope = np.asarray(input_scope)
    query = np.asarray(query)

    expected_scope = np.arange(B + 1, dtype=np.int64) * (N // B)
    if (
        x.shape == (N, D)
        and rel_weight.shape == (C, D)
        and input_scope.shape == (B + 1,)
        and np.array_equal(input_scope.astype(np.int64), expected_scope)
    ):
        out, _ = run_device(x, rel_weight, bias, query)
        return out
    return _numpy_fallback(x, rel_weight, bias, input_scope, query)


# revision 14
# speedup vs baseline: 1.1349x; 1.1305x over previous
"""Trainium2 Bass kernel for bag-level attention (ragged_sequence).

Math (per bag b over its 16 sentences i):
    att_i  = <x_i, rel[q_i]>
    w      = softmax(att) within bag
    logits = (sum_i w_i x_i) @ rel.T + bias

Key identity: logits[b] = sum_i w_i S[i,:] + bias with S = x @ rel.T, so x is
read from HBM exactly once. The kernel is HBM-bound; x is streamed as plain
fp16 (the 2e-2 correctness gate leaves ~50x margin at fp16's ~3.5e-4 rel err),
which halves traffic vs an fp16 hi+lo split. The one-hot query mask is built
on device from a 64 KB fp16 query row instead of streaming an 8 MB mask.

Measured engine notes: DVE/ScalarE cost ~1.2ns per FREE element regardless of
partition count; all-fp32 vector ops (PSUM reads included) are the fastest
variant; fp16 x fp16 and fp16 x fp32 tensor_tensor run SLOWER. GpSimd
partition_broadcast is ~1.4us per [C, 512] op regardless of dtype.

Device layout (per core, rows = N/8 sentences, ch=512-sentence chunks):
    st[64, ch]  = relT(64-col zero-padded).T @ xT chunk      (PSUM, 6 matmuls)
    oh[64, sc]  = one-hot of query: bcast(q) == iota          (GpSimd + DVE)
    sm          = st * oh (fp16 out)                          (DVE)
    att[1, ch]  = ones53.T @ sm                               (PE column sum)
    e           = exp(att)                                    (ScalarE)
    zac row     = windowed reduce_16(e)                       (DVE)
    ebs[53, ch] = partition_broadcast(e)                      (GpSimd)
    ltz rows    = windowed reduce_16(st * ebs)                (DVE x2)
  Every 128 bags (deferred one chunk so the PE never waits on the fresh
  reduce): pt[128, 54] = ltz_blk.T @ bt53 + zac_blk.T @ btz gives
  pt[:, c] = lu[c] + z*bias[c] and pt[:, 53] = z; then
  logits_block = pt[:, 0:53] * (1/z) per-partition.
"""

import os
from contextlib import ExitStack

import numpy as np

import concourse.bass as bass
import concourse.tile as tile
from concourse import bacc, library_config, mybir
from concourse.bass_utils import run_bass_kernel_spmd

# Problem constants (hardcoded per spec nn_Attention_85478439125349)
N = 262144
B = 16384
D = 768
C = 53
BAG = 16
N_CORES = 8
ROWS = N // N_CORES          # 32768 sentences per core
BAGS = B // N_CORES          # 2048 bags per core
KCH = D // 128               # 6 contraction chunks
F32 = mybir.dt.float32
F16 = mybir.dt.float16


def build_nc(rows: int, sc: int = 2048, ch: int = 512) -> bass.Bass:
    """Build the per-core Bass program for `rows` sentences (bags of BAG)."""
    assert rows % sc == 0 and sc % ch == 0 and ch % BAG == 0
    bags = rows // BAG
    n_sc = rows // sc          # superchunks (DMA granularity)
    n_ch = sc // ch            # compute chunks per superchunk
    chb = ch // BAG            # bags per compute chunk (32)

    nc = bacc.Bacc()
    # x fp16, partition-major packed per superchunk so each partition's
    # DMA run is KCH*sc contiguous elements: xt3[p, isc, k, j] =
    # xT[128k+p, isc*sc+j]
    xt3 = nc.declare_dram_parameter(
        "xt3", [128, rows // sc, KCH, sc], F16, isOutput=False
    )
    # query as fp16 row (values 0..52, exact in fp16)
    qft = nc.declare_dram_parameter("qft", [1, rows], F16, isOutput=False)
    # relT zero-padded to 64 output columns, split by contraction chunk
    relt = nc.declare_dram_parameter("relt", [128, KCH, 64], F16, isOutput=False)
    iotat = nc.declare_dram_parameter("iotat", [64, 1], F32, isOutput=False)
    onest = nc.declare_dram_parameter("onest", [C, 1], F16, isOutput=False)
    # Augmented transpose operands: pt = lu_blk.T @ bt53 + z_blk.T @ btz
    # emits lu + z*bias in cols 0:53 and z itself in col 53.
    bt53m = nc.declare_dram_parameter("bt53m", [C, C + 1], F32, isOutput=False)
    btzm = nc.declare_dram_parameter("btzm", [1, C + 1], F32, isOutput=False)
    out = nc.declare_dram_parameter("out", [bags, C], F32, isOutput=True)

    with tile.TileContext(nc) as tc, ExitStack() as ctx:
        consts = ctx.enter_context(tc.tile_pool(name="consts", bufs=1))
        xpool = ctx.enter_context(tc.tile_pool(name="xpool", bufs=4))
        ohpool = ctx.enter_context(tc.tile_pool(name="ohpool", bufs=2))
        work = ctx.enter_context(tc.tile_pool(name="work", bufs=3))
        psum = ctx.enter_context(tc.tile_pool(name="psum", bufs=2, space="PSUM"))

        # --- constants ---
        relt_sb = consts.tile([128, KCH, 64], F16)
        nc.sync.dma_start(out=relt_sb, in_=relt[:, :, :])
        iota_sb = consts.tile([64, 1], F32)
        nc.sync.dma_start(out=iota_sb, in_=iotat[:, :])
        ones_sb = consts.tile([C, 1], F16)
        nc.sync.dma_start(out=ones_sb, in_=onest[:, :])
        bt53_sb = consts.tile([C, C + 1], F32)
        nc.sync.dma_start(out=bt53_sb, in_=bt53m[:, :])
        btz_sb = consts.tile([1, C + 1], F32)
        nc.sync.dma_start(out=btz_sb, in_=btzm[:, :])
        nc.gpsimd.load_library(library_config.attn)
        # ltz accumulates unnormalized logits^T, zac the bag sums z
        ltz = consts.tile([C, bags], F32)
        zac = consts.tile([1, bags], F32)
        logits_sb = consts.tile([128, bags // 128, C], F32)

        # Chunk-granular software pipeline: per-engine instruction streams
        # are in-order, so chunk i's late stage (which waits on the
        # GpSimd/ACT softmax chain) is emitted only after chunk i+1's early
        # stages - otherwise each engine head-of-line blocks on the chain.
        pend_a = {}  # chunk -> (st, sm): waiting for att/exp stage
        pend_b = {}  # chunk -> (st, ebs): waiting for weighted-sum stage
        pend_c: list = []  # completed 128-bag blocks awaiting transpose

        def stage_mid(i):
            st, sm = pend_a.pop(i)
            att = psum.tile([1, ch], F32, tag="att", bufs=2)
            nc.tensor.matmul(att, lhsT=ones_sb, rhs=sm)
            e = work.tile([1, ch], F32, tag="e", bufs=4)
            nc.scalar.activation(e, att, mybir.ActivationFunctionType.Exp)
            ob = i * chb
            nc.vector.reduce_sum(
                zac[:, ob : ob + chb],
                e.rearrange("p (b j) -> p b j", j=BAG),
                axis=mybir.AxisListType.X,
            )
            ebs = work.tile([C, ch], F32, tag="ebs", bufs=8)
            nc.gpsimd.partition_broadcast(ebs, e, channels=C)
            pend_b[i] = (st, ebs)

        def stage_late(i):
            st, ebs = pend_b.pop(i)
            w = work.tile([C, ch], F32, tag="w", bufs=3)
            nc.vector.tensor_mul(w, st[0:C, :], ebs)
            ob = i * chb
            nc.vector.reduce_sum(
                ltz[:, ob : ob + chb],
                w.rearrange("p (b j) -> p b j", j=BAG),
                axis=mybir.AxisListType.X,
            )
            if (i + 1) * chb % 128 == 0:
                pend_c.append(((i + 1) * chb) // 128 - 1)

        def stage_fin():
            # transpose a completed 128-bag block to [bags, C] and
            # normalize; deferred one chunk so the PE never waits on the
            # just-issued reduce
            t = pend_c.pop(0)
            blk = slice(t * 128, (t + 1) * 128)
            pt = psum.tile([128, C + 1], F32, tag="pt", bufs=2)
            nc.tensor.matmul(
                pt, lhsT=ltz[:, blk], rhs=bt53_sb, start=True, stop=False
            )
            nc.tensor.matmul(
                pt, lhsT=zac[:, blk], rhs=btz_sb, start=False, stop=True
            )
            rzc = work.tile([128, 1], F32, tag="rzc")
            nc.vector.reciprocal(rzc, pt[:, C : C + 1])
            nc.vector.tensor_scalar_mul(
                out=logits_sb[:, t, :], in0=pt[:, 0:C], scalar1=rzc
            )

        x_sb = oh_sb = None
        n_total = n_sc * n_ch
        for i in range(n_total):
            isc, ic = divmod(i, n_ch)
            if ic == 0:
                x_sb = xpool.tile([128, KCH, sc], F16, bufs=4)
                kh = KCH // 2
                nc.sync.dma_start(out=x_sb[:, 0:kh, :], in_=xt3[:, isc, 0:kh, :])
                nc.sync.dma_start(
                    out=x_sb[:, kh:KCH, :], in_=xt3[:, isc, kh:KCH, :]
                )
                qf_sb = xpool.tile([1, sc], F16, tag="qf", bufs=4)
                nc.sync.dma_start(out=qf_sb, in_=qft[:, isc * sc : (isc + 1) * sc])
                qb_sb = ohpool.tile([64, sc], F16, tag="qb", bufs=2)
                nc.gpsimd.partition_broadcast(qb_sb, qf_sb, channels=64)
                oh_sb = ohpool.tile([64, sc], F16, tag="oh", bufs=2)
                nc.vector.tensor_scalar(
                    out=oh_sb,
                    in0=qb_sb,
                    scalar1=iota_sb,
                    scalar2=None,
                    op0=mybir.AluOpType.is_equal,
                )

            cs = slice(ic * ch, (ic + 1) * ch)
            st = psum.tile([64, ch], F32, tag="st", bufs=4)
            for k in range(KCH):
                nc.tensor.matmul(
                    st,
                    lhsT=relt_sb[:, k, :],
                    rhs=x_sb[:, k, cs],
                    start=(k == 0),
                    stop=(k == KCH - 1),
                )
            sm = work.tile([C, ch], F16, tag="sm", bufs=4)
            nc.vector.tensor_mul(sm, st[0:C, :], oh_sb[0:C, cs])
            pend_a[i] = (st, sm)
            if pend_c:
                stage_fin()
            if i > 0:
                stage_mid(i - 1)
            if i > 1:
                stage_late(i - 2)
        stage_mid(n_total - 1)
        stage_late(n_total - 2)
        stage_late(n_total - 1)
        while pend_c:
            stage_fin()
        nc.sync.dma_start(
            out=out.rearrange("(t p) c -> p t c", p=128), in_=logits_sb
        )
    return nc


_NC_CACHE: dict = {}


def _get_nc(rows: int) -> bass.Bass:
    if rows not in _NC_CACHE:
        nc = build_nc(rows)
        nc.finalize()
        _NC_CACHE[rows] = nc
    return _NC_CACHE[rows]


def _numpy_fallback(x, rel_weight, bias, input_scope, query):
    """Pure-numpy replication of the reference for non-uniform bag layouts."""
    n = x.shape[0]
    num_bags = input_scope.shape[0] - 1
    seg = np.searchsorted(input_scope[1:], np.arange(n), side="right")
    att = np.einsum("nd,nd->n", x, rel_weight[query]).astype(np.float32)
    valid = seg < num_bags
    segv = seg[valid]
    attv = att[valid]
    m = np.full(num_bags, -np.inf, dtype=np.float32)
    np.maximum.at(m, segv, attv)
    e = np.zeros(n, dtype=np.float32)
    e[valid] = np.exp(attv - m[segv])
    z = np.zeros(num_bags, dtype=np.float32)
    np.add.at(z, segv, e[valid])
    w = np.zeros(n, dtype=np.float32)
    nz = z[segv] != 0
    w_valid = np.zeros(segv.shape[0], dtype=np.float32)
    w_valid[nz] = e[valid][nz] / z[segv][nz]
    w[valid] = w_valid
    repre = np.zeros((num_bags, x.shape[1]), dtype=np.float32)
    np.add.at(repre, segv, (x[valid] * w[valid][:, None]).astype(np.float32))
    return repre @ rel_weight.T + bias


def _pack_x(xt_h, sc):
    """[D, rows] fp16 -> [128, rows//sc, KCH, sc] so each partition's
    per-superchunk DMA run (KCH*sc elements) is contiguous."""
    rows = xt_h.shape[1]
    v = xt_h.reshape(KCH, 128, rows // sc, sc)
    return np.ascontiguousarray(v.transpose(1, 2, 0, 3))


def _prepare_in_maps(x, rel_weight, bias, query, sc=2048):
    relp = np.zeros((D, 64), dtype=np.float16)
    relp[:, :C] = rel_weight.T.astype(np.float16)
    relt = np.ascontiguousarray(
        relp.reshape(KCH, 128, 64).transpose(1, 0, 2)
    )
    iotat = np.arange(64, dtype=np.float32).reshape(64, 1)
    onest = np.ones((C, 1), dtype=np.float16)
    bt53m = np.zeros((C, C + 1), dtype=np.float32)
    bt53m[np.arange(C), np.arange(C)] = 1.0
    btzm = np.zeros((1, C + 1), dtype=np.float32)
    btzm[0, :C] = bias.astype(np.float32)
    btzm[0, C] = 1.0
    q = query.astype(np.float16).reshape(1, -1)
    in_maps = []
    for c in range(N_CORES):
        lo_r, hi_r = c * ROWS, (c + 1) * ROWS
        xh = x[lo_r:hi_r].astype(np.float16)
        in_maps.append(
            {
                "xt3": _pack_x(np.ascontiguousarray(xh.T), sc),
                "qft": np.ascontiguousarray(q[:, lo_r:hi_r]),
                "relt": relt,
                "iotat": iotat,
                "onest": onest,
                "bt53m": bt53m,
                "btzm": btzm,
            }
        )
    return in_maps


def run_device(x, rel_weight, bias, query, trace=False, **kwargs):
    nc = _get_nc(ROWS)
    in_maps = _prepare_in_maps(x, rel_weight, bias, query)
    res = run_bass_kernel_spmd(
        nc, in_maps, core_ids=list(range(N_CORES)), trace=trace, **kwargs
    )
    outs = [np.asarray(r["out"]) for r in res.results]
    return np.concatenate(outs, axis=0), res


def kernel(x, rel_weight, bias, input_scope, query):
    x = np.asarray(x, dtype=np.float32)
    rel_weight = np.asarray(rel_weight, dtype=np.float32)
    bias = np.asarray(bias, dtype=np.float32)
    input_scope = np.asarray(input_scope)
    query = np.asarray(query)

    expected_scope = np.arange(B + 1, dtype=np.int64) * (N // B)
    if (
        x.shape == (N, D)
        and rel_weight.shape == (C, D)
        and input_scope.shape == (B + 1,)
        and np.array_equal(input_scope.astype(np.int64), expected_scope)
    ):
        out, _ = run_device(x, rel_weight, bias, query)
        return out
    return _numpy_fallback(x, rel_weight, bias, input_scope, query)


# revision 15
# speedup vs baseline: 1.2105x; 1.0666x over previous
"""Trainium2 Bass kernel for bag-level attention (ragged_sequence).

Math (per bag b over its 16 sentences i):
    att_i  = <x_i, rel[q_i]>
    w      = softmax(att) within bag
    logits = (sum_i w_i x_i) @ rel.T + bias

Key identity: logits[b] = sum_i w_i S[i,:] + bias with S = x @ rel.T, so x is
read from HBM exactly once. The kernel is HBM-bound; x is streamed as plain
fp16 (the 2e-2 correctness gate leaves ~50x margin at fp16's ~3.5e-4 rel err),
which halves traffic vs an fp16 hi+lo split. The one-hot query mask is built
on device from a 64 KB fp16 query row instead of streaming an 8 MB mask.

Measured engine notes: DVE/ScalarE cost ~1.2ns per FREE element regardless of
partition count; all-fp32 vector ops (PSUM reads included) are the fastest
variant; fp16 x fp16 and fp16 x fp32 tensor_tensor run SLOWER. GpSimd
partition_broadcast is ~1.4us per [C, 512] op regardless of dtype.

Device layout (per core, rows = N/8 sentences, ch=512-sentence chunks):
    st[64, ch]  = relT(64-col zero-padded).T @ xT chunk      (PSUM, 6 matmuls)
    oh[64, sc]  = one-hot of query: bcast(q) == iota          (GpSimd + DVE)
    sm          = st * oh (fp16 out)                          (DVE)
    att[1, ch]  = ones53.T @ sm                               (PE column sum)
    e           = exp(att)                                    (ScalarE)
    zac row     = windowed reduce_16(e)                       (DVE)
    ebs[53, ch] = partition_broadcast(e)                      (GpSimd)
    ltz rows    = windowed reduce_16(st * ebs)                (DVE x2)
  Every 128 bags (deferred one chunk so the PE never waits on the fresh
  reduce): pt[128, 54] = ltz_blk.T @ bt53 + zac_blk.T @ btz gives
  pt[:, c] = lu[c] + z*bias[c] and pt[:, 53] = z; then
  logits_block = pt[:, 0:53] * (1/z) per-partition.
"""

import os
from contextlib import ExitStack

import numpy as np

import concourse.bass as bass
import concourse.tile as tile
from concourse import bacc, library_config, mybir
from concourse.bass_utils import run_bass_kernel_spmd

# Problem constants (hardcoded per spec nn_Attention_85478439125349)
N = 262144
B = 16384
D = 768
C = 53
BAG = 16
N_CORES = 8
ROWS = N // N_CORES          # 32768 sentences per core
BAGS = B // N_CORES          # 2048 bags per core
KCH = D // 128               # 6 contraction chunks
F32 = mybir.dt.float32
F16 = mybir.dt.float16


def build_nc(rows: int, sc: int = 2048, ch: int = 512) -> bass.Bass:
    """Build the per-core Bass program for `rows` sentences (bags of BAG)."""
    assert rows % sc == 0 and sc % ch == 0 and ch % BAG == 0
    bags = rows // BAG
    n_sc = rows // sc          # superchunks (DMA granularity)
    n_ch = sc // ch            # compute chunks per superchunk
    chb = ch // BAG            # bags per compute chunk (32)

    nc = bacc.Bacc()
    # x fp16, partition-major packed per superchunk so each partition's
    # DMA run is KCH*sc contiguous elements: xt3[p, isc, k, j] =
    # xT[128k+p, isc*sc+j]
    xt3 = nc.declare_dram_parameter(
        "xt3", [128, rows // sc, KCH, sc], F16, isOutput=False
    )
    # query as fp16 row (values 0..52, exact in fp16)
    qft = nc.declare_dram_parameter("qft", [1, rows], F16, isOutput=False)
    # relT zero-padded to 64 output columns, split by contraction chunk
    relt = nc.declare_dram_parameter("relt", [128, KCH, 64], F16, isOutput=False)
    iotat = nc.declare_dram_parameter("iotat", [64, 1], F32, isOutput=False)
    onest = nc.declare_dram_parameter("onest", [C, 1], F16, isOutput=False)
    # Augmented transpose operands: pt = lu_blk.T @ bt53 + z_blk.T @ btz
    # emits lu + z*bias in cols 0:53 and z itself in col 53.
    bt53m = nc.declare_dram_parameter("bt53m", [C, C + 1], F32, isOutput=False)
    btzm = nc.declare_dram_parameter("btzm", [1, C + 1], F32, isOutput=False)
    out = nc.declare_dram_parameter("out", [bags, C], F32, isOutput=True)

    with tile.TileContext(nc) as tc, ExitStack() as ctx:
        consts = ctx.enter_context(tc.tile_pool(name="consts", bufs=1))
        xpool = ctx.enter_context(tc.tile_pool(name="xpool", bufs=4))
        ohpool = ctx.enter_context(tc.tile_pool(name="ohpool", bufs=2))
        work = ctx.enter_context(tc.tile_pool(name="work", bufs=3))
        psum = ctx.enter_context(tc.tile_pool(name="psum", bufs=2, space="PSUM"))

        # --- constants ---
        relt_sb = consts.tile([128, KCH, 64], F16)
        nc.sync.dma_start(out=relt_sb, in_=relt[:, :, :])
        iota_sb = consts.tile([64, 1], F32)
        nc.sync.dma_start(out=iota_sb, in_=iotat[:, :])
        ones_sb = consts.tile([C, 1], F16)
        nc.sync.dma_start(out=ones_sb, in_=onest[:, :])
        bt53_sb = consts.tile([C, C + 1], F32)
        nc.sync.dma_start(out=bt53_sb, in_=bt53m[:, :])
        btz_sb = consts.tile([1, C + 1], F32)
        nc.sync.dma_start(out=btz_sb, in_=btzm[:, :])
        nc.gpsimd.load_library(library_config.attn)
        # ltz accumulates unnormalized logits^T, zac the bag sums z
        ltz = consts.tile([C, bags], F32)
        zac = consts.tile([1, bags], F32)
        logits_sb = consts.tile([128, bags // 128, C], F32)

        # Chunk-granular software pipeline: per-engine instruction streams
        # are in-order, so chunk i's late stage (which waits on the
        # GpSimd/ACT softmax chain) is emitted only after chunk i+1's early
        # stages - otherwise each engine head-of-line blocks on the chain.
        pend_a = {}  # chunk -> (st, sm): waiting for att/exp stage
        pend_b = {}  # chunk -> (st, ebs): waiting for weighted-sum stage
        pend_c: list = []  # completed 128-bag blocks awaiting transpose

        def stage_mid(i):
            st, sm = pend_a.pop(i)
            att = psum.tile([1, ch], F32, tag="att", bufs=2)
            nc.tensor.matmul(att, lhsT=ones_sb, rhs=sm)
            e = work.tile([1, ch], F32, tag="e")
            nc.scalar.activation(e, att, mybir.ActivationFunctionType.Exp)
            ob = i * chb
            nc.vector.reduce_sum(
                zac[:, ob : ob + chb],
                e.rearrange("p (b j) -> p b j", j=BAG),
                axis=mybir.AxisListType.X,
            )
            ebs = work.tile([C, ch], F32, tag="ebs")
            nc.gpsimd.partition_broadcast(ebs, e, channels=C)
            pend_b[i] = (st, ebs)

        def stage_late(i):
            st, ebs = pend_b.pop(i)
            w = work.tile([C, ch], F32, tag="w")
            nc.vector.tensor_mul(w, st[0:C, :], ebs)
            ob = i * chb
            nc.vector.reduce_sum(
                ltz[:, ob : ob + chb],
                w.rearrange("p (b j) -> p b j", j=BAG),
                axis=mybir.AxisListType.X,
            )
            if (i + 1) * chb % 128 == 0:
                pend_c.append(((i + 1) * chb) // 128 - 1)

        def stage_fin():
            # transpose a completed 128-bag block to [bags, C] and
            # normalize; deferred one chunk so the PE never waits on the
            # just-issued reduce
            t = pend_c.pop(0)
            blk = slice(t * 128, (t + 1) * 128)
            pt = psum.tile([128, C + 1], F32, tag="pt", bufs=2)
            nc.tensor.matmul(
                pt, lhsT=ltz[:, blk], rhs=bt53_sb, start=True, stop=False
            )
            nc.tensor.matmul(
                pt, lhsT=zac[:, blk], rhs=btz_sb, start=False, stop=True
            )
            rzc = work.tile([128, 1], F32, tag="rzc")
            nc.vector.reciprocal(rzc, pt[:, C : C + 1])
            nc.vector.tensor_scalar_mul(
                out=logits_sb[:, t, :], in0=pt[:, 0:C], scalar1=rzc
            )

        x_sb = oh_sb = None
        n_total = n_sc * n_ch
        for i in range(n_total):
            isc, ic = divmod(i, n_ch)
            if ic == 0:
                x_sb = xpool.tile([128, KCH, sc], F16, bufs=3)
                nc.sync.dma_start(out=x_sb, in_=xt3[:, isc, :, :])
                qf_sb = xpool.tile([1, sc], F16, tag="qf", bufs=3)
                nc.sync.dma_start(out=qf_sb, in_=qft[:, isc * sc : (isc + 1) * sc])
                qb_sb = ohpool.tile([64, sc], F16, tag="qb", bufs=2)
                nc.gpsimd.partition_broadcast(qb_sb, qf_sb, channels=64)
                oh_sb = ohpool.tile([64, sc], F16, tag="oh", bufs=2)
                nc.vector.tensor_scalar(
                    out=oh_sb,
                    in0=qb_sb,
                    scalar1=iota_sb,
                    scalar2=None,
                    op0=mybir.AluOpType.is_equal,
                )

            cs = slice(ic * ch, (ic + 1) * ch)
            st = psum.tile([64, ch], F32, tag="st", bufs=4)
            for k in range(KCH):
                nc.tensor.matmul(
                    st,
                    lhsT=relt_sb[:, k, :],
                    rhs=x_sb[:, k, cs],
                    start=(k == 0),
                    stop=(k == KCH - 1),
                )
            sm = work.tile([C, ch], F16, tag="sm")
            nc.vector.tensor_mul(sm, st[0:C, :], oh_sb[0:C, cs])
            pend_a[i] = (st, sm)
            if pend_c:
                stage_fin()
            if i > 0:
                stage_mid(i - 1)
            if i > 1:
                stage_late(i - 2)
        stage_mid(n_total - 1)
        stage_late(n_total - 2)
        stage_late(n_total - 1)
        while pend_c:
            stage_fin()
        nc.sync.dma_start(
            out=out.rearrange("(t p) c -> p t c", p=128), in_=logits_sb
        )
    return nc


_NC_CACHE: dict = {}


def _get_nc(rows: int) -> bass.Bass:
    if rows not in _NC_CACHE:
        nc = build_nc(rows)
        nc.finalize()
        _NC_CACHE[rows] = nc
    return _NC_CACHE[rows]


def _numpy_fallback(x, rel_weight, bias, input_scope, query):
    """Pure-numpy replication of the reference for non-uniform bag layouts."""
    n = x.shape[0]
    num_bags = input_scope.shape[0] - 1
    seg = np.searchsorted(input_scope[1:], np.arange(n), side="right")
    att = np.einsum("nd,nd->n", x, rel_weight[query]).astype(np.float32)
    valid = seg < num_bags
    segv = seg[valid]
    attv = att[valid]
    m = np.full(num_bags, -np.inf, dtype=np.float32)
    np.maximum.at(m, segv, attv)
    e = np.zeros(n, dtype=np.float32)
    e[valid] = np.exp(attv - m[segv])
    z = np.zeros(num_bags, dtype=np.float32)
    np.add.at(z, segv, e[valid])
    w = np.zeros(n, dtype=np.float32)
    nz = z[segv] != 0
    w_valid = np.zeros(segv.shape[0], dtype=np.float32)
    w_valid[nz] = e[valid][nz] / z[segv][nz]
    w[valid] = w_valid
    repre = np.zeros((num_bags, x.shape[1]), dtype=np.float32)
    np.add.at(repre, segv, (x[valid] * w[valid][:, None]).astype(np.float32))
    return repre @ rel_weight.T + bias


def _pack_x(xt_h, sc):
    """[D, rows] fp16 -> [128, rows//sc, KCH, sc] so each partition's
    per-superchunk DMA run (KCH*sc elements) is contiguous."""
    rows = xt_h.shape[1]
    v = xt_h.reshape(KCH, 128, rows // sc, sc)
    return np.ascontiguousarray(v.transpose(1, 2, 0, 3))


def _prepare_in_maps(x, rel_weight, bias, query, sc=2048):
    relp = np.zeros((D, 64), dtype=np.float16)
    relp[:, :C] = rel_weight.T.astype(np.float16)
    relt = np.ascontiguousarray(
        relp.reshape(KCH, 128, 64).transpose(1, 0, 2)
    )
    iotat = np.arange(64, dtype=np.float32).reshape(64, 1)
    onest = np.ones((C, 1), dtype=np.float16)
    bt53m = np.zeros((C, C + 1), dtype=np.float32)
    bt53m[np.arange(C), np.arange(C)] = 1.0
    btzm = np.zeros((1, C + 1), dtype=np.float32)
    btzm[0, :C] = bias.astype(np.float32)
    btzm[0, C] = 1.0
    q = query.astype(np.float16).reshape(1, -1)
    in_maps = []
    for c in range(N_CORES):
        lo_r, hi_r = c * ROWS, (c + 1) * ROWS
        xh = x[lo_r:hi_r].astype(np.float16)
        in_maps.append(
            {
                "xt3": _pack_x(np.ascontiguousarray(xh.T), sc),
                "qft": np.ascontiguousarray(q[:, lo_r:hi_r]),
                "relt": relt,
                "iotat": iotat,
                "onest": onest,
                "bt53m": bt53m,
                "btzm": btzm,
            }
        )
    return in_maps


def run_device(x, rel_weight, bias, query, trace=False, **kwargs):
    nc = _get_nc(ROWS)
    in_maps = _prepare_in_maps(x, rel_weight, bias, query)
    res = run_bass_kernel_spmd(
        nc, in_maps, core_ids=list(range(N_CORES)), trace=trace, **kwargs
    )
    outs = [np.asarray(r["out"]) for r in res.results]
    return np.concatenate(outs, axis=0), res


def kernel(x, rel_weight, bias, input_scope, query):
    x = np.asarray(x, dtype=np.float32)
    rel_weight = np.asarray(rel_weight, dtype=np.float32)
    bias = np.asarray(bias, dtype=np.float32)
    input_scope = np.asarray(input_scope)
    query = np.asarray(query)

    expected_scope = np.arange(B + 1, dtype=np.int64) * (N // B)
    if (
        x.shape == (N, D)
        and rel_weight.shape == (C, D)
        and input_scope.shape == (B + 1,)
        and np.array_equal(input_scope.astype(np.int64), expected_scope)
    ):
        out, _ = run_device(x, rel_weight, bias, query)
        return out
    return _numpy_fallback(x, rel_weight, bias, input_scope, query)


# revision 16
# speedup vs baseline: 1.2142x; 1.0030x over previous
"""Trainium2 Bass kernel for bag-level attention (ragged_sequence).

Math (per bag b over its 16 sentences i):
    att_i  = <x_i, rel[q_i]>
    w      = softmax(att) within bag
    logits = (sum_i w_i x_i) @ rel.T + bias

Key identity: logits[b] = sum_i w_i S[i,:] + bias with S = x @ rel.T, so x is
read from HBM exactly once. The kernel is HBM-bound; x is streamed as plain
fp16 (the 2e-2 correctness gate leaves ~50x margin at fp16's ~3.5e-4 rel err),
which halves traffic vs an fp16 hi+lo split. The one-hot query mask is built
on device from a 64 KB fp16 query row instead of streaming an 8 MB mask.

Measured engine notes: DVE/ScalarE cost ~1.2ns per FREE element regardless of
partition count; all-fp32 vector ops (PSUM reads included) are the fastest
variant; fp16 x fp16 and fp16 x fp32 tensor_tensor run SLOWER. GpSimd
partition_broadcast is ~1.4us per [C, 512] op regardless of dtype.

Device layout (per core, rows = N/8 sentences, ch=512-sentence chunks):
    st[64, ch]  = relT(64-col zero-padded).T @ xT chunk      (PSUM, 6 matmuls)
    oh[64, sc]  = one-hot of query: bcast(q) == iota          (GpSimd + DVE)
    sm          = st * oh (fp16 out)                          (DVE)
    att[1, ch]  = ones53.T @ sm                               (PE column sum)
    e           = exp(att)                                    (ScalarE)
    zac row     = windowed reduce_16(e)                       (DVE)
    ebs[53, ch] = partition_broadcast(e)                      (GpSimd)
    ltz rows    = windowed reduce_16(st * ebs)                (DVE x2)
  Every 128 bags (deferred one chunk so the PE never waits on the fresh
  reduce): pt[128, 54] = ltz_blk.T @ bt53 + zac_blk.T @ btz gives
  pt[:, c] = lu[c] + z*bias[c] and pt[:, 53] = z; then
  logits_block = pt[:, 0:53] * (1/z) per-partition.
"""

import os
from contextlib import ExitStack

import numpy as np

import concourse.bass as bass
import concourse.tile as tile
from concourse import bacc, library_config, mybir
from concourse.bass_utils import run_bass_kernel_spmd

# Problem constants (hardcoded per spec nn_Attention_85478439125349)
N = 262144
B = 16384
D = 768
C = 53
BAG = 16
N_CORES = 8
ROWS = N // N_CORES          # 32768 sentences per core
BAGS = B // N_CORES          # 2048 bags per core
KCH = D // 128               # 6 contraction chunks
F32 = mybir.dt.float32
F16 = mybir.dt.float16


def build_nc(rows: int, sc: int = 2048, ch: int = 512) -> bass.Bass:
    """Build the per-core Bass program for `rows` sentences (bags of BAG)."""
    assert rows % sc == 0 and sc % ch == 0 and ch % BAG == 0
    bags = rows // BAG
    n_sc = rows // sc          # superchunks (DMA granularity)
    n_ch = sc // ch            # compute chunks per superchunk
    chb = ch // BAG            # bags per compute chunk (32)

    nc = bacc.Bacc()
    # x fp16, partition-major packed per superchunk so each partition's
    # DMA run is KCH*sc contiguous elements: xt3[p, isc, k, j] =
    # xT[128k+p, isc*sc+j]
    xt3 = nc.declare_dram_parameter(
        "xt3", [128, rows // sc, KCH, sc], F16, isOutput=False
    )
    # query as fp16 row (values 0..52, exact in fp16)
    qft = nc.declare_dram_parameter("qft", [1, rows], F16, isOutput=False)
    # relT zero-padded to 64 output columns, split by contraction chunk
    relt = nc.declare_dram_parameter("relt", [128, KCH, 64], F16, isOutput=False)
    iotat = nc.declare_dram_parameter("iotat", [64, 1], F32, isOutput=False)
    onest = nc.declare_dram_parameter("onest", [C, 1], F16, isOutput=False)
    # Augmented transpose operands: pt = lu_blk.T @ bt53 + z_blk.T @ btz
    # emits lu + z*bias in cols 0:53 and z itself in col 53.
    bt53m = nc.declare_dram_parameter("bt53m", [C, C + 1], F32, isOutput=False)
    btzm = nc.declare_dram_parameter("btzm", [1, C + 1], F32, isOutput=False)
    out = nc.declare_dram_parameter("out", [bags, C], F32, isOutput=True)

    with tile.TileContext(nc) as tc, ExitStack() as ctx:
        consts = ctx.enter_context(tc.tile_pool(name="consts", bufs=1))
        xpool = ctx.enter_context(tc.tile_pool(name="xpool", bufs=4))
        ohpool = ctx.enter_context(tc.tile_pool(name="ohpool", bufs=2))
        work = ctx.enter_context(tc.tile_pool(name="work", bufs=3))
        psum = ctx.enter_context(tc.tile_pool(name="psum", bufs=2, space="PSUM"))

        # --- constants ---
        relt_sb = consts.tile([128, KCH, 64], F16)
        nc.sync.dma_start(out=relt_sb, in_=relt[:, :, :])
        iota_sb = consts.tile([64, 1], F32)
        nc.sync.dma_start(out=iota_sb, in_=iotat[:, :])
        ones_sb = consts.tile([C, 1], F16)
        nc.sync.dma_start(out=ones_sb, in_=onest[:, :])
        bt53_sb = consts.tile([C, C + 1], F32)
        nc.sync.dma_start(out=bt53_sb, in_=bt53m[:, :])
        btz_sb = consts.tile([1, C + 1], F32)
        nc.sync.dma_start(out=btz_sb, in_=btzm[:, :])
        nc.gpsimd.load_library(library_config.attn)
        # ltz accumulates unnormalized logits^T, zac the bag sums z
        ltz = consts.tile([C, bags], F32)
        zac = consts.tile([1, bags], F32)
        logits_sb = consts.tile([128, bags // 128, C], F32)

        # Chunk-granular software pipeline: per-engine instruction streams
        # are in-order, so chunk i's late stage (which waits on the
        # GpSimd/ACT softmax chain) is emitted only after chunk i+1's early
        # stages - otherwise each engine head-of-line blocks on the chain.
        pend_a = {}  # chunk -> (st, sm): waiting for att/exp stage
        pend_b = {}  # chunk -> (st, ebs): waiting for weighted-sum stage

        def stage_mid(i):
            st, sm = pend_a.pop(i)
            att = psum.tile([1, ch], F32, tag="att", bufs=2)
            nc.tensor.matmul(att, lhsT=ones_sb, rhs=sm)
            e = work.tile([1, ch], F16, tag="e")
            nc.scalar.activation(e, att, mybir.ActivationFunctionType.Exp)
            ob = i * chb
            nc.vector.reduce_sum(
                zac[:, ob : ob + chb],
                e.rearrange("p (b j) -> p b j", j=BAG),
                axis=mybir.AxisListType.X,
            )
            # fp16 broadcast halves the GpSimd bytes; the downstream w
            # multiply reads st(PSUM fp32) x ebs(fp16), the fast mixed form
            ebs = work.tile([C, ch], F16, tag="ebs")
            nc.gpsimd.partition_broadcast(ebs, e, channels=C)
            pend_b[i] = (st, ebs)

        def stage_late(i):
            st, ebs = pend_b.pop(i)
            w = work.tile([C, ch], F32, tag="w")
            nc.vector.tensor_mul(w, st[0:C, :], ebs)
            ob = i * chb
            nc.vector.reduce_sum(
                ltz[:, ob : ob + chb],
                w.rearrange("p (b j) -> p b j", j=BAG),
                axis=mybir.AxisListType.X,
            )
            # once a 128-bag block is complete, transpose it to [bags, C]
            # and normalize; overlaps with the remaining chunks
            if (i + 1) * chb % 128 == 0:
                t = ((i + 1) * chb) // 128 - 1
                blk = slice(t * 128, (t + 1) * 128)
                pt = psum.tile([128, C + 1], F32, tag="pt", bufs=2)
                nc.tensor.matmul(
                    pt, lhsT=ltz[:, blk], rhs=bt53_sb, start=True, stop=False
                )
                nc.tensor.matmul(
                    pt, lhsT=zac[:, blk], rhs=btz_sb, start=False, stop=True
                )
                rzc = work.tile([128, 1], F32, tag="rzc")
                nc.vector.reciprocal(rzc, pt[:, C : C + 1])
                nc.vector.tensor_scalar_mul(
                    out=logits_sb[:, t, :], in0=pt[:, 0:C], scalar1=rzc
                )

        x_sb = oh_sb = None
        n_total = n_sc * n_ch
        for i in range(n_total):
            isc, ic = divmod(i, n_ch)
            if ic == 0:
                x_sb = xpool.tile([128, KCH, sc], F16, bufs=3)
                nc.sync.dma_start(out=x_sb, in_=xt3[:, isc, :, :])
                qf_sb = xpool.tile([1, sc], F16, tag="qf", bufs=3)
                nc.sync.dma_start(out=qf_sb, in_=qft[:, isc * sc : (isc + 1) * sc])
                qb_sb = ohpool.tile([64, sc], F16, tag="qb", bufs=2)
                nc.gpsimd.partition_broadcast(qb_sb, qf_sb, channels=64)
                oh_sb = ohpool.tile([64, sc], F16, tag="oh", bufs=2)
                nc.vector.tensor_scalar(
                    out=oh_sb,
                    in0=qb_sb,
                    scalar1=iota_sb,
                    scalar2=None,
                    op0=mybir.AluOpType.is_equal,
                )

            cs = slice(ic * ch, (ic + 1) * ch)
            st = psum.tile([64, ch], F32, tag="st", bufs=4)
            for k in range(KCH):
                nc.tensor.matmul(
                    st,
                    lhsT=relt_sb[:, k, :],
                    rhs=x_sb[:, k, cs],
                    start=(k == 0),
                    stop=(k == KCH - 1),
                )
            sm = work.tile([C, ch], F16, tag="sm")
            nc.vector.tensor_mul(sm, st[0:C, :], oh_sb[0:C, cs])
            pend_a[i] = (st, sm)
            if i > 0:
                stage_mid(i - 1)
            if i > 1:
                stage_late(i - 2)
        stage_mid(n_total - 1)
        stage_late(n_total - 2)
        stage_late(n_total - 1)
        nc.sync.dma_start(
            out=out.rearrange("(t p) c -> p t c", p=128), in_=logits_sb
        )
    return nc


_NC_CACHE: dict = {}


def _get_nc(rows: int) -> bass.Bass:
    if rows not in _NC_CACHE:
        nc = build_nc(rows)
        nc.finalize()
        _NC_CACHE[rows] = nc
    return _NC_CACHE[rows]


def _numpy_fallback(x, rel_weight, bias, input_scope, query):
    """Pure-numpy replication of the reference for non-uniform bag layouts."""
    n = x.shape[0]
    num_bags = input_scope.shape[0] - 1
    seg = np.searchsorted(input_scope[1:], np.arange(n), side="right")
    att = np.einsum("nd,nd->n", x, rel_weight[query]).astype(np.float32)
    valid = seg < num_bags
    segv = seg[valid]
    attv = att[valid]
    m = np.full(num_bags, -np.inf, dtype=np.float32)
    np.maximum.at(m, segv, attv)
    e = np.zeros(n, dtype=np.float32)
    e[valid] = np.exp(attv - m[segv])
    z = np.zeros(num_bags, dtype=np.float32)
    np.add.at(z, segv, e[valid])
    w = np.zeros(n, dtype=np.float32)
    nz = z[segv] != 0
    w_valid = np.zeros(segv.shape[0], dtype=np.float32)
    w_valid[nz] = e[valid][nz] / z[segv][nz]
    w[valid] = w_valid
    repre = np.zeros((num_bags, x.shape[1]), dtype=np.float32)
    np.add.at(repre, segv, (x[valid] * w[valid][:, None]).astype(np.float32))
    return repre @ rel_weight.T + bias


def _pack_x(xt_h, sc):
    """[D, rows] fp16 -> [128, rows//sc, KCH, sc] so each partition's
    per-superchunk DMA run (KCH*sc elements) is contiguous."""
    rows = xt_h.shape[1]
    v = xt_h.reshape(KCH, 128, rows // sc, sc)
    return np.ascontiguousarray(v.transpose(1, 2, 0, 3))


def _prepare_in_maps(x, rel_weight, bias, query, sc=2048):
    relp = np.zeros((D, 64), dtype=np.float16)
    relp[:, :C] = rel_weight.T.astype(np.float16)
    relt = np.ascontiguousarray(
        relp.reshape(KCH, 128, 64).transpose(1, 0, 2)
    )
    iotat = np.arange(64, dtype=np.float32).reshape(64, 1)
    onest = np.ones((C, 1), dtype=np.float16)
    bt53m = np.zeros((C, C + 1), dtype=np.float32)
    bt53m[np.arange(C), np.arange(C)] = 1.0
    btzm = np.zeros((1, C + 1), dtype=np.float32)
    btzm[0, :C] = bias.astype(np.float32)
    btzm[0, C] = 1.0
    q = query.astype(np.float16).reshape(1, -1)
    in_maps = []
    for c in range(N_CORES):
        lo_r, hi_r = c * ROWS, (c + 1) * ROWS
        xh = x[lo_r:hi_r].astype(np.float16)
        in_maps.append(
            {
                "xt3": _pack_x(np.ascontiguousarray(xh.T), sc),
                "qft": np.ascontiguousarray(q[:, lo_r:hi_r]),
                "relt": relt,
                "iotat": iotat,
                "onest": onest,
                "bt53m": bt53m,
                "btzm": btzm,
            }
        )
    return in_maps


def run_device(x, rel_weight, bias, query, trace=False, **kwargs):
    nc = _get_nc(ROWS)
    in_maps = _prepare_in_maps(x, rel_weight, bias, query)
    res = run_bass_kernel_spmd(
        nc, in_maps, core_ids=list(range(N_CORES)), trace=trace, **kwargs
    )
    outs = [np.asarray(r["out"]) for r in res.results]
    return np.concatenate(outs, axis=0), res


def kernel(x, rel_weight, bias, input_scope, query):
    x = np.asarray(x, dtype=np.float32)
    rel_weight = np.asarray(rel_weight, dtype=np.float32)
    bias = np.asarray(bias, dtype=np.float32)
    input_scope = np.asarray(input_scope)
    query = np.asarray(query)

    expected_scope = np.arange(B + 1, dtype=np.int64) * (N // B)
    if (
        x.shape == (N, D)
        and rel_weight.shape == (C, D)
        and input_scope.shape == (B + 1,)
        and np.array_equal(input_scope.astype(np.int64), expected_scope)
    ):
        out, _ = run_device(x, rel_weight, bias, query)
        return out
    return _numpy_fallback(x, rel_weight, bias, input_scope, query)
